# revision 1
# baseline (speedup 1.0000x reference)
"""Causal MLA attention kernel for 8 Trainium2 NeuronCores.

Sharding: core c = (batch b = c//4) x (head-group g = c%4, 4 heads each).
Each core computes its batch's q/k/v projections for its 4 heads, RoPE
(folded into the low-rank compression weights), latent attention, the
decompression, and a partial output projection against its slice of Wo.
The host sums the 4 head-group partials per batch and adds bo.

Device layout (validated in numpy + CoreSim against the reference):
  * Per-head q/k/v columns permuted to [even dims, odd dims] so RoPE
    becomes  qc = W1.T @ (qT*cosT) + W2.T @ (qT*sinT)  with
    W1 = Wqc[perm], W2 = [Wqc[odd]; -Wqc[even]] - no rotation step.
  * Softmax without row-max: scores are in [-0.8, 0.8] for this model;
    normalization is deferred via a leading ones column in V (row 0 of
    the attention PSUM accumulates sum(exp)).
  * Latents stored bf16 (7.7e-5 model error), everything else float32r
    (full PE rate at moving dim >= 256).
  * Projections run k, v, then q with attention interleaved after each
    q chunk so the ACT-bound exp stream overlaps projection matmuls.
"""
import numpy as np

B, L, D, H = 2, 2048, 2048, 16
HD, LD = 128, 32
HPC = 4            # heads per core
JW = HPC * HD      # 512 local projection width
NJT = JW // 128    # 4 j tiles
NDT = D // 128     # 16 d (contraction) tiles
NCH = 4            # l chunks
CH = L // NCH      # 512
NKT = L // 128     # 16 k tiles
LH = L // 2        # half length (one pass)
SCALE = 1.0 / np.sqrt(np.float32(LD))
N_CORES = 8

_perm = np.concatenate([np.arange(0, HD, 2), np.arange(1, HD, 2)])


# --------------------------------------------------------------------------
# host-side prep / gather
# --------------------------------------------------------------------------

def _host_prep(inputs, core):
    f = np.float32
    b, g = core // 4, core % 4
    cols = np.concatenate([(4 * g + h) * HD + _perm for h in range(HPC)])
    m = {}
    m['xt'] = np.ascontiguousarray(inputs['x'][b].T.astype(f))          # (D, L)
    for nm in ('q', 'k', 'v'):
        m['w' + nm] = np.ascontiguousarray(inputs['W' + nm][:, cols].astype(f))
        m['b' + nm] = np.ascontiguousarray(
            inputs['b' + nm][cols].astype(f).reshape(NJT, 128).T)        # (128, 4)
    for nm in ('q', 'k'):
        Wc = inputs['W' + nm + 'c'].astype(f)
        m['w1' + nm] = np.ascontiguousarray(Wc[_perm])                   # (128, 32)
        m['w2' + nm] = np.ascontiguousarray(
            np.concatenate([Wc[1::2], -Wc[0::2]]))                       # (128, 32)
        m['b' + nm + 'c'] = np.ascontiguousarray(
            inputs['b' + nm + 'c'].astype(f)[:, None])                   # (32, 1)
    m['wvc'] = np.ascontiguousarray(inputs['Wvc'].astype(f)[_perm])      # (128, 32)
    m['bvc4'] = np.ascontiguousarray(
        np.broadcast_to(np.tile(inputs['bvc'].astype(f), NJT), (128, 128)))
    m['wd1'] = np.ascontiguousarray(
        np.concatenate([np.zeros((1, HD), f), inputs['Wd'].astype(f)]))  # (33, 128)
    m['bd'] = np.ascontiguousarray(inputs['bd'].astype(f)[:, None])      # (128, 1)
    m['wo'] = np.ascontiguousarray(
        inputs['Wo'][4 * g * HD:(4 * g + HPC) * HD].astype(f))           # (512, D)
    ct = inputs['cos'].astype(f).T
    st = inputs['sin'].astype(f).T
    m['cost'] = np.ascontiguousarray(np.concatenate([ct, ct]))           # (128, L)
    m['sint'] = np.ascontiguousarray(np.concatenate([st, st]))           # (128, L)
    p = np.arange(128)[:, None]
    j = np.arange(CH)[None, :]
    m['masks'] = np.stack(
        [(128 * mm + p <= j).astype(f) for mm in range(4)])              # (4,128,512)
    m['ones1'] = np.ones((128, NKT, 1), f)
    return m


def _gather(results, inputs):
    out = np.zeros((B, L, D), np.float32)
    for core in range(N_CORES):
        out[core // 4] += results[core]['out']
    out += inputs['bo'].astype(np.float32)
    return out


# --------------------------------------------------------------------------
# device program (SPMD - identical on all 8 cores)
# --------------------------------------------------------------------------

def build_nc():
    import concourse.bass as bass
    import concourse.mybir as mybir
    import concourse.tile as tile
    from concourse import bacc

    f32 = mybir.dt.float32
    f32r = mybir.dt.float32r
    bf16 = mybir.dt.bfloat16
    ACT = mybir.ActivationFunctionType

    nc = bacc.Bacc("TRN2", target_bir_lowering=False)

    dram = {}
    def din(name, shape, dt=f32):
        dram[name] = nc.dram_tensor(name, list(shape), dt, kind="ExternalInput")
    din('xt', (D, L), f32r)
    for nm in ('q', 'k', 'v'):
        din('w' + nm, (D, JW), f32r); din('b' + nm, (128, NJT))
    for nm in ('q', 'k'):
        din('w1' + nm, (128, LD), f32r); din('w2' + nm, (128, LD), f32r)
        din('b' + nm + 'c', (LD, 1))
    din('wvc', (128, LD), f32r); din('bvc4', (128, 128))
    din('wd1', (33, HD), f32r); din('bd', (128, 1))
    din('wo', (JW, D), f32r)
    din('cost', (128, L)); din('sint', (128, L))
    din('masks', (4, 128, CH))
    din('ones1', (128, NKT, 1), f32r)
    out_dram = nc.dram_tensor('out', [L, D], f32, kind="ExternalOutput")
    decb = nc.dram_tensor('decb', [HPC, 128, L], f32r)   # dec bounce buffer

    def mm(out, lhsT, rhs, **kw):
        nc.tensor.matmul(out, lhsT, rhs, **kw)

    with tile.TileContext(nc) as tc, \
         tc.tile_pool(name="persist", bufs=1) as persist:

        small = {}
        for name in ('w1q', 'w2q', 'w1k', 'w2k', 'wvc', 'bvc4',
                     'bq', 'bk', 'bv', 'bqc', 'bkc', 'bd'):
            dt_ = f32r if name in ('w1q', 'w2q', 'w1k', 'w2k', 'wvc') else f32
            t = persist.tile(list(dram[name].shape), dt_, tag=name,
                             name=name + '_sb')
            nc.sync.dma_start(out=t[:], in_=dram[name][:])
            small[name] = t
        wd1_sb = persist.tile([33, HD], f32r, tag="wd1")
        nc.sync.dma_start(out=wd1_sb[:], in_=dram['wd1'][:])
        mask4_sb = persist.tile([128, 4, CH], f32, tag="mask4")
        nc.sync.dma_start(out=mask4_sb[:],
                          in_=dram['masks'][:].rearrange("m p j -> p m j"))
        mask_sb = [mask4_sb[:, mi, :] for mi in range(4)]

        qc_sb = [persist.tile([LD, L], bf16, tag=f"qc{h}", name=f"qc{h}_sb")
                 for h in range(HPC)]
        kc_sb = [persist.tile([LD, L], bf16, tag=f"kc{h}", name=f"kc{h}_sb")
                 for h in range(HPC)]
        vc_sb = [persist.tile([128, NKT, LD + 1], f32r, tag=f"vc{h}",
                              name=f"vc{h}_sb") for h in range(HPC)]
        for h in range(HPC):
            nc.sync.dma_start(out=vc_sb[h][:, :, 0:1],
                              in_=dram['ones1'][:])      # leading ones column

        with (
            tc.tile_pool(name="xt", bufs=1) as xt_pool,
            tc.tile_pool(name="wst", bufs=5) as w_pool,
            tc.tile_pool(name="pj", bufs=2) as pj_pool,
            tc.tile_pool(name="prod", bufs=2) as prod_pool,
            tc.tile_pool(name="cs", bufs=1) as cs_pool,
            tc.tile_pool(name="exp", bufs=3) as exp_pool,
            tc.tile_pool(name="att", bufs=1) as att_pool,
            tc.tile_pool(name="dst", bufs=1) as dst_pool,
            tc.tile_pool(name="pspj", bufs=2, space="PSUM") as pspj_pool,
            tc.tile_pool(name="psqc", bufs=1, space="PSUM") as psqc_pool,
            tc.tile_pool(name="psS", bufs=2, space="PSUM") as psS_pool,
            tc.tile_pool(name="psA", bufs=2, space="PSUM") as psA_pool,
        ):
            def norm_dec(c, pair, psA):
                for h in pair:
                    rs = att_pool.tile([1, CH], f32, tag="rs")
                    nc.vector.reciprocal(rs[:], psA[h][0:1, :])
                    rsb = att_pool.tile([LD + 1, CH], f32, tag="rsb")
                    nc.gpsimd.partition_broadcast(rsb[:], rs[:])
                    at = att_pool.tile([LD + 1, CH], f32r, tag="at")
                    nc.vector.tensor_mul(at[:], psA[h], rsb[:])
                    psD = psS_pool.tile([128, CH], f32, tag="psS",
                                        name="psD_t")
                    mm(psD[:], wd1_sb[:], at[:], start=True, stop=True)
                    dst = dst_pool.tile([128, CH], f32r, tag="dst")
                    nc.vector.tensor_scalar_add(dst[:], psD[:],
                                                small['bd'][:])
                    nc.sync.dma_start(
                        out=decb[h, :, c * CH:(c + 1) * CH], in_=dst[:])

            def attn_chunk(c):
                nkt = 4 * (c + 1)
                pending = None
                for hp in range(2):
                    pair = (2 * hp, 2 * hp + 1)
                    psA = {h: psA_pool.tile([LD + 1, CH], f32, tag="psA",
                                            name="psA_t") for h in pair}
                    for kt in range(nkt):
                        for h in pair:
                            psS = psS_pool.tile([128, CH], f32, tag="psS",
                                                name="psS_t")
                            mm(psS[:],
                               kc_sb[h][:, kt * 128:(kt + 1) * 128],
                               qc_sb[h][:, c * CH:(c + 1) * CH],
                               start=True, stop=True)
                            ex = exp_pool.tile([128, CH], f32r, tag="ex")
                            nc.scalar.activation(ex[:], psS[:], ACT.Exp,
                                                 scale=float(SCALE))
                            if kt >= 4 * c:
                                nc.vector.tensor_mul(
                                    ex[:], ex[:], mask_sb[kt - 4 * c])
                            mm(psA[h], vc_sb[h][:, kt, :], ex[:],
                               start=(kt == 0), stop=(kt == nkt - 1))
                        if kt == 1 and pending is not None:
                            norm_dec(c, *pending)
                            pending = None
                    pending = (pair, psA)
                if pending is not None:
                    norm_dec(c, *pending)

            for lpass in range(2):
                l0 = lpass * LH
                # interleave first-proj weight DMAs with xt so the first
                # accumulation chain starts as soon as tile 0 lands
                wt_k = []
                xt_sb = []
                for blk in range(NDT // 2):
                    if blk % 2 == 0:
                        t = w_pool.tile([128, 4, JW], f32r, tag="w",
                                        name="w_sb")
                        nc.sync.dma_start(
                            out=t[:],
                            in_=dram['wk'][blk * 256:(blk + 2) * 256, :]
                            .rearrange("(b p) j -> p b j", p=128))
                        wt_k.append(t)
                    x = xt_pool.tile([128, 2, LH], f32r, tag=f"xt{blk}",
                                     name=f"xt{blk}_sb")
                    nc.sync.dma_start(
                        out=x[:],
                        in_=dram['xt'][blk * 256:(blk + 1) * 256, l0:l0 + LH]
                        .rearrange("(b p) l -> p b l", p=128))
                    xt_sb.append(x)
                cost_sb = cs_pool.tile([128, LH], f32, tag="cost")
                sint_sb = cs_pool.tile([128, LH], f32, tag="sint")
                nc.sync.dma_start(out=cost_sb[:], in_=dram['cost'][:, l0:l0 + LH])
                nc.sync.dma_start(out=sint_sb[:], in_=dram['sint'][:, l0:l0 + LH])

                for proj in ('k', 'v', 'q'):
                    if proj == 'k':
                        wt = wt_k
                    else:
                        wt = []
                        for wb in range(NDT // 4):
                            t = w_pool.tile([128, 4, JW], f32r, tag="w",
                                            name="w_sb")
                            nc.sync.dma_start(
                                out=t[:],
                                in_=dram['w' + proj][wb * 512:(wb + 1) * 512, :]
                                .rearrange("(b p) j -> p b j", p=128))
                            wt.append(t)
                    for ci in range(2):
                        c = 2 * lpass + ci
                        for jt in range(NJT):        # jt == head index
                            ps_p = pspj_pool.tile([128, CH], f32, tag="pj",
                                                  name="ps_p")
                            for dt in range(NDT):
                                mm(ps_p[:],
                                   wt[dt // 4][:, dt % 4,
                                               jt * 128:(jt + 1) * 128],
                                   xt_sb[dt // 2][:, dt % 2,
                                                  ci * CH:(ci + 1) * CH],
                                   start=(dt == 0), stop=(dt == NDT - 1))
                            pT = pj_pool.tile([128, CH], f32r, tag="pT")
                            nc.vector.tensor_scalar_add(
                                pT[:], ps_p[:], small['b' + proj][:, jt:jt + 1])
                            if proj != 'v':
                                pc = prod_pool.tile([128, CH], f32r, tag="pc")
                                ps_ = prod_pool.tile([128, CH], f32r, tag="ps")
                                nc.gpsimd.tensor_mul(
                                    pc[:], pT[:], cost_sb[:, ci * CH:(ci + 1) * CH])
                                nc.gpsimd.tensor_mul(
                                    ps_[:], pT[:], sint_sb[:, ci * CH:(ci + 1) * CH])
                                ps_qc = psqc_pool.tile([LD, CH], f32, tag="qc",
                                                       name="ps_qc")
                                mm(ps_qc[:], small['w1' + proj][:], pc[:],
                                   start=True, stop=False)
                                mm(ps_qc[:], small['w2' + proj][:], ps_[:],
                                   start=False, stop=True)
                                dstl = qc_sb if proj == 'q' else kc_sb
                                nc.vector.tensor_scalar_add(
                                    dstl[jt][:, c * CH:(c + 1) * CH], ps_qc[:],
                                    small['b' + proj + 'c'][:])
                            else:
                                ps_vc = psS_pool.tile([128, NJT, LD], f32,
                                                      tag="vc", name="ps_vc",
                                                      bufs=1)
                                for lt in range(NJT):
                                    mm(ps_vc[:, lt, :],
                                       pT[:, lt * 128:(lt + 1) * 128],
                                       small['wvc'][:],
                                       start=True, stop=True)
                                nc.vector.tensor_add(
                                    vc_sb[jt][:, c * NJT:(c + 1) * NJT, 1:],
                                    ps_vc[:],
                                    small['bvc4'][:].rearrange(
                                        "p (a b) -> p a b", a=NJT))
                        if proj == 'q':
                            attn_chunk(c)

        # ----------------- output projection (dec from DRAM) --------------
        with (
            tc.tile_pool(name="wo", bufs=1) as wo_pool,
            tc.tile_pool(name="dect", bufs=3) as dect_pool,
            tc.tile_pool(name="ot", bufs=2) as ot_pool,
            tc.tile_pool(name="psO", bufs=3, space="PSUM") as psO_pool,
        ):
            wo_sb = []
            for hb in range(HPC):
                t = wo_pool.tile([128, D], f32r, tag=f"wo{hb}", name="wo_sb")
                nc.sync.dma_start(out=t[:],
                                  in_=dram['wo'][hb * 128:(hb + 1) * 128, :])
                wo_sb.append(t)
            for lt in range(16):
                dect = dect_pool.tile([128, HPC, 128], f32r, tag="dect",
                                      name="dect_t")
                nc.sync.dma_start(
                    out=dect[:],
                    in_=decb[:, :, lt * 128:(lt + 1) * 128].rearrange(
                        "h p l -> p h l"))
                orow = ot_pool.tile([128, D], f32, tag="ot", bufs=2)
                for dc in range(4):
                    ps_o = psO_pool.tile([128, CH], f32, tag="psO",
                                         name="ps_o")
                    for h in range(HPC):
                        mm(ps_o[:], dect[:, h, :],
                           wo_sb[h][:, dc * CH:(dc + 1) * CH],
                           start=(h == 0), stop=(h == HPC - 1))
                    nc.vector.tensor_copy(orow[:, dc * CH:(dc + 1) * CH],
                                          ps_o[:])
                nc.sync.dma_start(
                    out=out_dram[lt * 128:(lt + 1) * 128, :], in_=orow[:])

    nc.compile()
    return nc


# --------------------------------------------------------------------------
# entry point
# --------------------------------------------------------------------------

_CACHE = {}


def _get_nc():
    if 'nc' not in _CACHE:
        _CACHE['nc'] = build_nc()
    return _CACHE['nc']


def kernel(**inputs):
    from concourse.bass_utils import run_bass_kernel_spmd
    nc = _get_nc()
    in_maps = [_host_prep(inputs, c) for c in range(N_CORES)]
    res = run_bass_kernel_spmd(nc, in_maps, core_ids=list(range(N_CORES)))
    return _gather(res.results, inputs)

